# revision 40
# baseline (speedup 1.0000x reference)
"""Trainium2 Bass kernel for CropConLoss (supervised-contrastive style loss).

Contract: kernel(**inputs) takes the FULL unsharded inputs
(protos [64,128] f32, proj2/proj3 [4096,128] f32, target2/target3 [4096] i64)
and returns the FULL output (scalar f32 mean loss), running the compute on
8 NeuronCores via bass_utils.run_bass_kernel_spmd.

Strategy (data-parallel over the M=8192 rows of feats, ACT-roofline design):
  - Host sorts the 8192 rows by class label and l2-normalizes them (and the
    protos) in f32, so the device needs no sqrt/reciprocal and a single
    constant exp scale of 1/tau.
  - Each core owns 1024 query rows. Layout is [query-partition, key-free]:
    per q-tile (128 queries) the stationary operand is the query block of
    keysT (bf16) and the 8192 keys stream through the PE in 512-col chunks
    into a 2-deep PSUM ring of [128, 2048] f32 tiles.
  - exp runs on ACT over the [128, 2048] PSUM chunks into bf16 SBUF tiles;
    ACT is the roofline engine (~60us of exp at 1 elem/cycle/lane).
  - v2 over the 87.9us baseline: the whole schedule is arranged so ACT
    starts as early as possible and never has tail work:
      * keysT is DMAed as 8 independent [128,1024] pieces in priority
        order (protosT first, then k0..k7, then masks/epilogue constants),
        so the first matmuls/exps start as soon as piece 0 lands instead
        of waiting for a monolithic 1MB keysA transfer.
      * both ACT tables (Exp and Ln) are warmed at kernel start, moving
        the 1.3us Ln ACT_TABLE_LOAD from the tail into the startup DMA
        shadow.
      * the proto-similarity block (matmul + exp + one-hot/weighted
        selects) runs BEFORE the main loop in a dedicated 1-bank PSUM
        pool, entirely inside the startup window where PE/ACT/DVE would
        otherwise idle waiting for keys.
      * the diagonal term exp(|q|^2/tau) is precomputed on HOST from the
        bf16-rounded features (dsubh input), eliminating the mdiag mask
        multiply+reduce per tile on DVE.
      * band-window mask multiplies run on the otherwise-idle GpSimd
        engine; DVE keeps only the free-axis reduces.
      * row-sums are split: chunks with ch==3 or t==7 use ACT accum_out
        (hidden RAA ~0.3us each, 11 total), the other 21 use DVE reduces,
        keeping both engines just under the exp roofline.
      * the epilogue is emitted as small dependency-staged ops so
        everything except the final ~6 instructions overlaps the last
        exp chunks; the tail after the last RAA is ~2us.
  - Epilogue: den = (rowsum - diag)*fwinv + dprot, num = own + nprot,
    loss partial = sum(ln den - ln num) via accum_out + ones-matmul.
  - Host sums the 8 partials and divides by 8192. No device collectives.

Known pitfalls encoded here: tensor_tensor_reduce passes CoreSim but
kills the device on this toolchain (split into tensor_tensor+reduce_sum);
GpSimd tensor_reduce cannot reduce the free axis; DVE reduce throughput
does not double for bf16, but bf16 inputs still reduce with f32
accumulation (verified: rel err ~7e-6).
"""

import sys
import types

sys.path.insert(0, "/opt/trn_rl_repo")

import numpy as np

TAU = 0.1
EPS_FREQ = 1e-06
EPS_DENOM = 1e-12

N_CORES = 8
M = 8192          # total rows (2*4096)
D = 128           # feature dim
C = 64            # num classes
Q = M // N_CORES  # 1024 query rows per core
NQT = Q // 128    # 8 query tiles per core
CHUNK = 2048      # key chunk per ACT instruction
NCH = M // CHUNK  # 4 chunks per q-tile
KP = 1024         # keysT DMA piece width
NKP = M // KP     # 8 pieces
W = 512           # band window width (own-class mates live here)
QOFF = 256        # own queries sit at rolled cols [QOFF, QOFF+Q)
WMARG = 192       # window starts at q-tile start - WMARG


def _install_ntff_hook():
    """Shim antenv.axon_hooks (absent in this image) so trace=True works."""
    if "antenv.axon_hooks" in sys.modules:
        return
    try:
        if "/root/.axon_site" not in sys.path:
            sys.path.insert(0, "/root/.axon_site")
        import trn_agent_boot.trn_boot as tb

        hook = tb._ntff_profile_via_ctypes("/opt/axon/libaxon_pjrt.so")
        mod = types.ModuleType("antenv.axon_hooks")
        mod._hook = hook
        mod.get_axon_ntff_profile_hook = lambda: mod._hook
        mod.set_axon_ntff_profile_hook = lambda h: setattr(mod, "_hook", h)
        sys.modules["antenv.axon_hooks"] = mod
        import antenv

        antenv.axon_hooks = mod
    except Exception:
        pass


def build_nc():
    """Build and compile the single-core Bass program (same NEFF on all 8)."""
    import concourse.bass as bass  # noqa: F401
    import concourse.mybir as mybir
    import concourse.bacc as bacc
    from concourse import bass_isa, tile

    f32 = mybir.dt.float32
    bf16 = mybir.dt.bfloat16
    fp8 = mybir.dt.float8e4
    mult = mybir.AluOpType.mult
    add = mybir.AluOpType.add
    sub = mybir.AluOpType.subtract
    Act = mybir.ActivationFunctionType

    nc = bacc.Bacc("TRN2", target_bir_lowering=False, debug=False,
                   num_devices=N_CORES)

    d_pkq = nc.dram_tensor("pkq", [128, C + Q], bf16, kind="ExternalInput")
    d_kq0 = nc.dram_tensor("kq0", [128, 512], fp8, kind="ExternalInput")
    d_kq1 = nc.dram_tensor("kq1", [128, 512], fp8, kind="ExternalInput")
    d_kq2 = nc.dram_tensor("kq2", [128, 1024], fp8, kind="ExternalInput")
    d_kB1 = nc.dram_tensor("kB1", [128, 2048], fp8, kind="ExternalInput")
    d_kB21 = nc.dram_tensor("kB21", [128, 2048], fp8, kind="ExternalInput")
    d_kB22 = nc.dram_tensor("kB22", [128, 2048], fp8, kind="ExternalInput")
    d_ohpT = nc.dram_tensor("ohpT", [C, Q], bf16, kind="ExternalInput")
    d_wB = nc.dram_tensor("wB", [C, Q], bf16, kind="ExternalInput")
    d_mclass = nc.dram_tensor("mclass", [128, NQT, W], bf16,
                              kind="ExternalInput")
    d_fwinv = nc.dram_tensor("fwinv", [128, NQT], f32, kind="ExternalInput")
    d_dsubh = nc.dram_tensor("dsubh", [128, NQT], f32, kind="ExternalInput")
    d_out = nc.dram_tensor("out", [1, 1], f32, kind="ExternalOutput")

    with tile.TileContext(nc) as tc:
        with (
            tc.tile_pool(name="const", bufs=1) as cst,
            tc.tile_pool(name="etring", bufs=6) as etring,
        ):
            pkq = cst.tile([128, C + Q], bf16, tag="pkq")
            protosT = pkq[:, 0:C]
            kq = pkq[:, C:C + Q]
            kq0 = cst.tile([128, 512], fp8, tag="kq0")
            kq1 = cst.tile([128, 512], fp8, tag="kq1")
            kq2 = cst.tile([128, 1024], fp8, tag="kq2")
            kB1 = cst.tile([128, 2048], fp8, tag="kB1")
            kB21 = cst.tile([128, 2048], fp8, tag="kB21")
            kB22 = cst.tile([128, 2048], fp8, tag="kB22")
            ohpT = cst.tile([C, Q], bf16, tag="ohpT")
            wB = cst.tile([C, Q], bf16, tag="wB")
            mclass = cst.tile([128, NQT, W], bf16, tag="mclass")
            fwinv = cst.tile([128, NQT], f32, tag="fwinv")
            dsubh = cst.tile([128, NQT], f32, tag="dsubh")

            # Exp and Ln live together only in the natural_log_exp_and_others
            # table set (id 6 in act_info.json); the automatic inserter picks
            # first-match sets per function and would reload on every
            # Exp<->Ln switch (1.3us each, one of them in the tail).
            # Pre-loading the combined set makes every later activation a
            # table hit, so the fixpoint pass inserts no further loads.
            nc.scalar.add_instruction(mybir.InstLoadActFuncSet(
                name=nc.get_next_instruction_name(), ins=[], outs=[],
                act_func_set_id=6))
            warm = cst.tile([1, 1], f32, tag="warm")
            nc.vector.memset(warm[:], 1.0)
            wj = cst.tile([1, 1], f32, tag="wj")
            nc.scalar.activation(wj[:], warm[:], Act.Exp)
            nc.scalar.activation(wj[:], warm[:], Act.Ln)

            # Input DMAs. Only 4 descriptors ride the sync queue (each desc
            # costs ~0.65us of serial issue time, so the critical-path
            # tensors get their own short queue): protos+queries first
            # (gate the proto block and every matmul stationary), then the
            # keys in consumption order. Everything else issues from the
            # scalar engine's HWDGE queue during ACT's idle startup window.
            nc.sync.dma_start(pkq[:], d_pkq[:])
            nc.sync.dma_start(kq0[:], d_kq0[:])
            nc.sync.dma_start(kq1[:], d_kq1[:])
            nc.sync.dma_start(kq2[:], d_kq2[:])
            nc.sync.dma_start(kB1[:], d_kB1[:])
            nc.sync.dma_start(kB21[:], d_kB21[:])
            nc.sync.dma_start(kB22[:], d_kB22[:])
            nc.sync.dma_start(mclass[:, 0:2], d_mclass[:, 0:2])
            nc.sync.dma_start(ohpT[:], d_ohpT[:])
            nc.sync.dma_start(wB[:], d_wB[:])
            nc.sync.dma_start(mclass[:, 2:NQT], d_mclass[:, 2:NQT])
            nc.sync.dma_start(fwinv[:], d_fwinv[:])
            nc.sync.dma_start(dsubh[:], d_dsubh[:])

            # accumulators / epilogue operands
            racc = [cst.tile([128, NQT], f32, tag=f"racc{i}",
                             name=f"racc{i}")
                    for i in range(NCH)]
            own = cst.tile([128, NQT], f32, tag="own")
            nprot = cst.tile([128, NQT], f32, tag="nprot")
            dprot = cst.tile([128, NQT], f32, tag="dprot")
            etpT = cst.tile([C, Q], bf16, tag="etpT")
            junkT = cst.tile([C, Q], f32, tag="junkT")
            nprB = cst.tile([C, 128, NQT], f32, tag="nprB")
            dprB = cst.tile([C, 128, NQT], f32, tag="dprB")
            junkw = cst.tile([128, W], bf16, tag="junkw")
            onescol = cst.tile([128, 1], f32, tag="onescol")
            nc.vector.memset(onescol[:], 1.0)
            dmy = cst.tile([128, 128], bf16, tag="dmy")
            nc.gpsimd.memset(dmy[:], 0.0)

            # ---- main loop: 8 q-tiles x 4 key chunks of 2048 ----
            # The warm-up matmuls (PE p-state ramp while DMAs stream) and
            # the proto-similarity block live in ordinary ring slots: a
            # separate PSUM pool would serialize the pool transition, so
            # the first key matmul would wait for the proto exp. As ring
            # tiles they pipeline like any other chunk.
            with tc.tile_pool(name="ring", bufs=2, space="PSUM") as ring:
                rw = ring.tile([128, CHUNK], f32, tag="ps", name="rw")
                for _ in range(4):
                    nc.tensor.matmul(rw[:, 0:128], dmy[:], dmy[:],
                                     start=True, stop=True)
                # proto similarity transposed: [class, query] so it costs
                # one stationary load + two matmuls instead of eight
                rp = ring.tile([128, CHUNK], f32, tag="ps", name="rp")
                nc.tensor.matmul(rp[0:C, 0:512], protosT[:], kq[:, 0:512],
                                 start=True, stop=True)
                nc.tensor.matmul(rp[0:C, 512:Q], protosT[:], kq[:, 512:Q],
                                 start=True, stop=True)
                nc.scalar.activation(etpT[:], rp[0:C, 0:Q], Act.Exp,
                                     scale=1.0 / TAU)
                # proto selects on the idle GpSimd engine: one-hot
                # (numerator) and 1/cls_freq-weighted (denominator)
                # partition sums. The all-reduce writes through a permuted
                # AP so the [q=(t,p)] result lands in [p-major, t] memory
                # order; the reshape to [128, NQT] is then a contiguous
                # DMA partition scatter.
                nc.gpsimd.tensor_tensor(junkT[:], etpT[:], ohpT[:], op=mult)
                nc.gpsimd.partition_all_reduce(nprB[:].transpose([0, 2, 1]),
                                               junkT[:], channels=C,
                                               reduce_op=bass_isa.ReduceOp.add)
                nc.gpsimd.tensor_tensor(junkT[:], etpT[:], wB[:], op=mult)
                nc.gpsimd.partition_all_reduce(dprB[:].transpose([0, 2, 1]),
                                               junkT[:], channels=C,
                                               reduce_op=bass_isa.ReduceOp.add)
                nc.sync.dma_start(nprot[:], nprB[0:1, :, :])
                nc.sync.dma_start(dprot[:], dprB[0:1, :, :])

                for t in range(NQT):
                    for ch in range(NCH):
                        ps = ring.tile([128, CHUNK], f32, tag="ps")
                        for j in range(CHUNK // 512):
                            cb = CHUNK * ch + 512 * j
                            if cb < 512:
                                mv, off = kq0, cb
                            elif cb < 1024:
                                mv, off = kq1, cb - 512
                            elif cb < 2048:
                                mv, off = kq2, cb - 1024
                            elif cb < 4096:
                                mv, off = kB1, cb - 2048
                            elif cb < 6144:
                                mv, off = kB21, cb - 4096
                            else:
                                mv, off = kB22, cb - 6144
                            nc.tensor.matmul(ps[:, 512 * j:512 * (j + 1)],
                                             kq[:, 128 * t:128 * t + 128],
                                             mv[:, off:off + 512],
                                             start=True, stop=True)
                        et = etring.tile([128, CHUNK], bf16, tag="et")
                        # split row-sum work: 11 chunks via ACT accum_out
                        # (hidden RAA ~0.3us each; t==7 fully on ACT keeps
                        # DVE off the kernel tail), 21 via DVE reduces
                        on_act = t == NQT - 1 or ch == NCH - 1
                        if on_act:
                            nc.scalar.activation(
                                et[:], ps[:], Act.Exp, scale=1.0 / TAU,
                                accum_out=racc[ch][:, t:t + 1])
                        else:
                            nc.scalar.activation(et[:], ps[:], Act.Exp,
                                                 scale=1.0 / TAU)
                        if ch == 0:
                            # band window: own-class sum (mask-mult + reduce
                            # on DVE; GpSimd owns the proto selects and an
                            # in-order cross-engine handoff here would stall
                            # the rowsum reduces behind it)
                            win = slice(64 + 128 * t, 64 + 128 * t + W)
                            nc.vector.tensor_tensor(junkw[:], et[:, win],
                                                    mclass[:, t], op=mult)
                            nc.vector.reduce_sum(own[:, t:t + 1], junkw[:],
                                                 axis=mybir.AxisListType.X)
                        if not on_act:
                            nc.vector.reduce_sum(racc[ch][:, t:t + 1], et[:],
                                                 axis=mybir.AxisListType.X)

            # ---- epilogue: dependency-staged so only the last few ops
            # trail the final exp chunk ----
            num = cst.tile([128, NQT], f32, tag="num")
            lbuf = cst.tile([128, NQT], f32, tag="lbuf")
            lbuf2 = cst.tile([128, NQT], f32, tag="lbuf2")
            l1 = cst.tile([128, 1], f32, tag="l1")
            l2 = cst.tile([128, 1], f32, tag="l2")
            # numerator path completes once t7ch0's band lands (early);
            # its reciprocal folds the two tail Lns into one:
            # sum(ln den - ln num) = sum(ln(den * (1/num)))
            rnum = cst.tile([128, NQT], f32, tag="rnum")
            nc.vector.tensor_tensor(num[:], own[:], nprot[:], op=add)
            nc.vector.reciprocal(rnum[:], num[:])

            # denominator row-sum partials fold in as each racc column set
            # completes; only "+ racc[3]" waits for the last RAA
            r01 = cst.tile([128, NQT], f32, tag="r01")
            r012 = cst.tile([128, NQT], f32, tag="r012")
            rs3p = cst.tile([128, NQT], f32, tag="rs3p")
            rs = cst.tile([128, NQT], f32, tag="rs")
            den = cst.tile([128, NQT], f32, tag="den")
            nc.vector.tensor_tensor(r01[:], racc[0][:], racc[1][:], op=add)
            nc.vector.tensor_tensor(r012[:], r01[:], racc[2][:], op=add)
            nc.vector.tensor_tensor(rs3p[:], r012[:], dsubh[:], op=sub)
            nc.vector.tensor_tensor(rs[:], rs3p[:], racc[3][:], op=add)
            nc.vector.tensor_tensor(den[:], rs[:], fwinv[:], op=mult)
            nc.vector.tensor_tensor(den[:], den[:], dprot[:], op=add)
            nc.vector.tensor_tensor(den[:], den[:], rnum[:], op=mult)
            nc.scalar.activation(lbuf2[:], den[:], Act.Ln, accum_out=l1[:])

            with tc.tile_pool(name="pf", bufs=1, space="PSUM") as pfp:
                pf = pfp.tile([1, 1], f32, tag="pf")
                nc.tensor.matmul(pf[:], onescol[:], l1[:],
                                 start=True, stop=True)
                res = cst.tile([1, 1], f32, tag="res")
                nc.vector.tensor_copy(res[:], pf[:])
                nc.sync.dma_start(d_out[:], res[:])

    nc.compile()
    return nc


def make_in_maps(protos, proj2, target2, proj3, target3):
    import ml_dtypes

    bf16 = ml_dtypes.bfloat16
    fp8 = ml_dtypes.float8_e4m3
    f32 = np.float32

    feats = np.concatenate([np.asarray(proj2, dtype=f32),
                            np.asarray(proj3, dtype=f32)], axis=0)
    labels = np.concatenate([np.asarray(target2), np.asarray(target3)],
                            axis=0).astype(np.int64)

    order = np.argsort(labels, kind="stable")
    fs = feats[order]
    ls = labels[order]
    nrm = np.sqrt((fs * fs).sum(axis=1, keepdims=True))
    fn = fs / np.maximum(nrm, f32(1e-12))

    counts = np.bincount(ls, minlength=C).astype(f32)
    # class-mates of any row must fit the [start-WMARG, end+WMARG] window
    assert counts.max() <= WMARG + 1, "class count exceeds band window"
    cls_freq = (counts + f32(1.0)) + f32(EPS_FREQ)
    cfr = (f32(1.0) / cls_freq).astype(f32)

    pr = np.asarray(protos, dtype=f32)
    pnrm = np.sqrt((pr * pr).sum(axis=1, keepdims=True))
    pn = pr / np.maximum(pnrm, f32(1e-12))
    protosT = np.ascontiguousarray(pn.T).astype(bf16)

    in_maps = []
    for c in range(N_CORES):
        roll = (Q * c - QOFF) % M
        idx = (np.arange(M) + roll) % M
        kf = fn[idx]
        kl = ls[idx]

        keysT = np.ascontiguousarray(kf.T).astype(bf16)
        keysT8 = np.ascontiguousarray(kf.T).astype(fp8)

        # host-side diagonal term: exp(q_bf16 . q_fp8 / tau) exactly as the
        # PE computes it (rounded inputs, f32 products/accumulate)
        qb = keysT[:, QOFF:QOFF + Q].astype(np.float64)
        q8 = keysT8[:, QOFF:QOFF + Q].astype(np.float64)
        ss = (qb * q8).sum(axis=0)                       # [Q]
        dsubh = np.exp(ss * (1.0 / TAU)).astype(f32)     # [Q]
        dsubh = np.ascontiguousarray(dsubh.reshape(NQT, 128).T)  # [128, NQT]

        mcls = np.zeros((128, NQT, W), dtype=bf16)
        fwinv = np.zeros((128, NQT), dtype=f32)
        for t in range(NQT):
            rows = kl[QOFF + 128 * t:QOFF + 128 * t + 128]
            win = kl[64 + 128 * t:64 + 128 * t + W]
            mc = rows[:, None] == win[None, :]
            mc[np.arange(128), np.arange(128) + WMARG] = False
            mcls[:, t, :] = mc.astype(bf16)
            fwinv[:, t] = cfr[rows]

        # transposed proto-select masks [class, query]
        qlab = kl[QOFF:QOFF + Q]                          # [Q]
        ohpT = (np.arange(C)[:, None] == qlab[None, :]).astype(bf16)
        wB = np.ascontiguousarray(
            np.broadcast_to(cfr[:, None], (C, Q))).astype(bf16)

        pkq = np.concatenate([protosT, keysT[:, QOFF:QOFF + Q]], axis=1)
        im = {
            "pkq": np.ascontiguousarray(pkq),
            "kq0": np.ascontiguousarray(keysT8[:, 0:512]),
            "kq1": np.ascontiguousarray(keysT8[:, 512:1024]),
            "kq2": np.ascontiguousarray(keysT8[:, 1024:2048]),
            "kB1": np.ascontiguousarray(keysT8[:, 2048:4096]),
            "kB21": np.ascontiguousarray(keysT8[:, 4096:6144]),
            "kB22": np.ascontiguousarray(keysT8[:, 6144:8192]),
            "mclass": mcls,
            "fwinv": fwinv,
            "dsubh": dsubh,
            "ohpT": np.ascontiguousarray(ohpT),
            "wB": wB,
        }
        in_maps.append(im)
    return in_maps


def run(in_maps, trace=False):
    _install_ntff_hook()
    from concourse import bass_utils

    nc = build_nc()
    res = bass_utils.run_bass_kernel_spmd(
        nc, in_maps, core_ids=list(range(N_CORES)), trace=trace)
    return res


def kernel(protos, proj2, target2, proj3, target3):
    in_maps = make_in_maps(protos, proj2, target2, proj3, target3)
    res = run(in_maps, trace=False)
    parts = [res.results[i]["out"][0, 0] for i in range(N_CORES)]
    total = np.sum(np.asarray(parts, dtype=np.float32))
    return np.asarray(total / np.float32(M), dtype=np.float32)


# revision 41
# speedup vs baseline: 1.0302x; 1.0302x over previous
"""Trainium2 Bass kernel for CropConLoss (supervised-contrastive style loss).

Contract: kernel(**inputs) takes the FULL unsharded inputs
(protos [64,128] f32, proj2/proj3 [4096,128] f32, target2/target3 [4096] i64)
and returns the FULL output (scalar f32 mean loss), running the compute on
8 NeuronCores via bass_utils.run_bass_kernel_spmd.

Strategy (data-parallel over the M=8192 rows of feats, ACT-roofline design):
  - Host sorts the 8192 rows by class label and l2-normalizes them (and the
    protos) in f32, so the device needs no sqrt/reciprocal and a single
    constant exp scale of 1/tau.
  - Each core owns 1024 query rows. Layout is [query-partition, key-free]:
    per q-tile (128 queries) the stationary operand is the query block of
    keysT (bf16) and the 8192 keys stream through the PE in 512-col chunks
    into a 2-deep PSUM ring of [128, 2048] f32 tiles.
  - exp runs on ACT over the [128, 2048] PSUM chunks into bf16 SBUF tiles;
    ACT is the roofline engine (~60us of exp at 1 elem/cycle/lane).
  - v2 over the 87.9us baseline: the whole schedule is arranged so ACT
    starts as early as possible and never has tail work:
      * keysT is DMAed as 8 independent [128,1024] pieces in priority
        order (protosT first, then k0..k7, then masks/epilogue constants),
        so the first matmuls/exps start as soon as piece 0 lands instead
        of waiting for a monolithic 1MB keysA transfer.
      * both ACT tables (Exp and Ln) are warmed at kernel start, moving
        the 1.3us Ln ACT_TABLE_LOAD from the tail into the startup DMA
        shadow.
      * the proto-similarity block (matmul + exp + one-hot/weighted
        selects) runs BEFORE the main loop in a dedicated 1-bank PSUM
        pool, entirely inside the startup window where PE/ACT/DVE would
        otherwise idle waiting for keys.
      * the diagonal term exp(|q|^2/tau) is precomputed on HOST from the
        bf16-rounded features (dsubh input), eliminating the mdiag mask
        multiply+reduce per tile on DVE.
      * band-window mask multiplies run on the otherwise-idle GpSimd
        engine; DVE keeps only the free-axis reduces.
      * row-sums are split: chunks with ch==3 or t==7 use ACT accum_out
        (hidden RAA ~0.3us each, 11 total), the other 21 use DVE reduces,
        keeping both engines just under the exp roofline.
      * the epilogue is emitted as small dependency-staged ops so
        everything except the final ~6 instructions overlaps the last
        exp chunks; the tail after the last RAA is ~2us.
  - Epilogue: den = (rowsum - diag)*fwinv + dprot, num = own + nprot,
    loss partial = sum(ln den - ln num) via accum_out + ones-matmul.
  - Host sums the 8 partials and divides by 8192. No device collectives.

Known pitfalls encoded here: tensor_tensor_reduce passes CoreSim but
kills the device on this toolchain (split into tensor_tensor+reduce_sum);
GpSimd tensor_reduce cannot reduce the free axis; DVE reduce throughput
does not double for bf16, but bf16 inputs still reduce with f32
accumulation (verified: rel err ~7e-6).
"""

import sys
import types

sys.path.insert(0, "/opt/trn_rl_repo")

import numpy as np

TAU = 0.1
EPS_FREQ = 1e-06
EPS_DENOM = 1e-12

N_CORES = 8
M = 8192          # total rows (2*4096)
D = 128           # feature dim
C = 64            # num classes
Q = M // N_CORES  # 1024 query rows per core
NQT = Q // 128    # 8 query tiles per core
CHUNK = 2048      # key chunk per ACT instruction
NCH = M // CHUNK  # 4 chunks per q-tile
KP = 1024         # keysT DMA piece width
NKP = M // KP     # 8 pieces
W = 512           # band window width (own-class mates live here)
QOFF = 256        # own queries sit at rolled cols [QOFF, QOFF+Q)
WMARG = 192       # window starts at q-tile start - WMARG


def _install_ntff_hook():
    """Shim antenv.axon_hooks (absent in this image) so trace=True works."""
    if "antenv.axon_hooks" in sys.modules:
        return
    try:
        if "/root/.axon_site" not in sys.path:
            sys.path.insert(0, "/root/.axon_site")
        import trn_agent_boot.trn_boot as tb

        hook = tb._ntff_profile_via_ctypes("/opt/axon/libaxon_pjrt.so")
        mod = types.ModuleType("antenv.axon_hooks")
        mod._hook = hook
        mod.get_axon_ntff_profile_hook = lambda: mod._hook
        mod.set_axon_ntff_profile_hook = lambda h: setattr(mod, "_hook", h)
        sys.modules["antenv.axon_hooks"] = mod
        import antenv

        antenv.axon_hooks = mod
    except Exception:
        pass


def build_nc():
    """Build and compile the single-core Bass program (same NEFF on all 8)."""
    import concourse.bass as bass  # noqa: F401
    import concourse.mybir as mybir
    import concourse.bacc as bacc
    from concourse import bass_isa, tile

    f32 = mybir.dt.float32
    bf16 = mybir.dt.bfloat16
    fp8 = mybir.dt.float8e4
    mult = mybir.AluOpType.mult
    add = mybir.AluOpType.add
    sub = mybir.AluOpType.subtract
    Act = mybir.ActivationFunctionType

    nc = bacc.Bacc("TRN2", target_bir_lowering=False, debug=False,
                   num_devices=N_CORES)

    d_pkq = nc.dram_tensor("pkq", [128, C + Q], fp8, kind="ExternalInput")
    d_kq0 = nc.dram_tensor("kq0", [128, 512], fp8, kind="ExternalInput")
    d_kq1 = nc.dram_tensor("kq1", [128, 512], fp8, kind="ExternalInput")
    d_kq2 = nc.dram_tensor("kq2", [128, 1024], fp8, kind="ExternalInput")
    d_kB1 = nc.dram_tensor("kB1", [128, 2048], fp8, kind="ExternalInput")
    d_kB21 = nc.dram_tensor("kB21", [128, 2048], fp8, kind="ExternalInput")
    d_kB22 = nc.dram_tensor("kB22", [128, 2048], fp8, kind="ExternalInput")
    d_ohpT = nc.dram_tensor("ohpT", [C, Q], bf16, kind="ExternalInput")
    d_wB = nc.dram_tensor("wB", [C, Q], bf16, kind="ExternalInput")
    d_mclass = nc.dram_tensor("mclass", [128, NQT, W], bf16,
                              kind="ExternalInput")
    d_fwinv = nc.dram_tensor("fwinv", [128, NQT], f32, kind="ExternalInput")
    d_dsubh = nc.dram_tensor("dsubh", [128, NQT], f32, kind="ExternalInput")
    d_out = nc.dram_tensor("out", [1, 1], f32, kind="ExternalOutput")

    with tile.TileContext(nc) as tc:
        with (
            tc.tile_pool(name="const", bufs=1) as cst,
            tc.tile_pool(name="etring", bufs=6) as etring,
        ):
            pkq = cst.tile([128, C + Q], fp8, tag="pkq")
            protosT = pkq[:, 0:C]
            kq = pkq[:, C:C + Q]
            kq0 = cst.tile([128, 512], fp8, tag="kq0")
            kq1 = cst.tile([128, 512], fp8, tag="kq1")
            kq2 = cst.tile([128, 1024], fp8, tag="kq2")
            kB1 = cst.tile([128, 2048], fp8, tag="kB1")
            kB21 = cst.tile([128, 2048], fp8, tag="kB21")
            kB22 = cst.tile([128, 2048], fp8, tag="kB22")
            ohpT = cst.tile([C, Q], bf16, tag="ohpT")
            wB = cst.tile([C, Q], bf16, tag="wB")
            mclass = cst.tile([128, NQT, W], bf16, tag="mclass")
            fwinv = cst.tile([128, NQT], f32, tag="fwinv")
            dsubh = cst.tile([128, NQT], f32, tag="dsubh")

            # Exp and Ln live together only in the natural_log_exp_and_others
            # table set (id 6 in act_info.json); the automatic inserter picks
            # first-match sets per function and would reload on every
            # Exp<->Ln switch (1.3us each, one of them in the tail).
            # Pre-loading the combined set makes every later activation a
            # table hit, so the fixpoint pass inserts no further loads.
            nc.scalar.add_instruction(mybir.InstLoadActFuncSet(
                name=nc.get_next_instruction_name(), ins=[], outs=[],
                act_func_set_id=6))
            warm = cst.tile([1, 1], f32, tag="warm")
            nc.vector.memset(warm[:], 1.0)
            wj = cst.tile([1, 1], f32, tag="wj")
            nc.scalar.activation(wj[:], warm[:], Act.Exp)
            nc.scalar.activation(wj[:], warm[:], Act.Ln)

            # Input DMAs. Only 4 descriptors ride the sync queue (each desc
            # costs ~0.65us of serial issue time, so the critical-path
            # tensors get their own short queue): protos+queries first
            # (gate the proto block and every matmul stationary), then the
            # keys in consumption order. Everything else issues from the
            # scalar engine's HWDGE queue during ACT's idle startup window.
            nc.sync.dma_start(pkq[:], d_pkq[:])
            nc.sync.dma_start(kq0[:], d_kq0[:])
            nc.sync.dma_start(kq1[:], d_kq1[:])
            nc.sync.dma_start(kq2[:], d_kq2[:])
            nc.sync.dma_start(kB1[:], d_kB1[:])
            nc.sync.dma_start(kB21[:], d_kB21[:])
            nc.sync.dma_start(kB22[:], d_kB22[:])
            nc.sync.dma_start(mclass[:, 0:2], d_mclass[:, 0:2])
            nc.sync.dma_start(ohpT[:], d_ohpT[:])
            nc.sync.dma_start(wB[:], d_wB[:])
            nc.sync.dma_start(mclass[:, 2:NQT], d_mclass[:, 2:NQT])
            nc.sync.dma_start(fwinv[:], d_fwinv[:])
            nc.sync.dma_start(dsubh[:], d_dsubh[:])

            # accumulators / epilogue operands
            racc = [cst.tile([128, NQT], f32, tag=f"racc{i}",
                             name=f"racc{i}")
                    for i in range(NCH)]
            own = cst.tile([128, NQT], f32, tag="own")
            nprot = cst.tile([128, NQT], f32, tag="nprot")
            dprot = cst.tile([128, NQT], f32, tag="dprot")
            etpT = cst.tile([C, Q], bf16, tag="etpT")
            junkT = cst.tile([C, Q], f32, tag="junkT")
            nprB = cst.tile([C, 128, NQT], f32, tag="nprB")
            dprB = cst.tile([C, 128, NQT], f32, tag="dprB")
            junkw = cst.tile([128, W], bf16, tag="junkw")
            onescol = cst.tile([128, 1], f32, tag="onescol")
            nc.vector.memset(onescol[:], 1.0)
            dmy = cst.tile([128, 128], bf16, tag="dmy")
            nc.gpsimd.memset(dmy[:], 0.0)

            # ---- main loop: 8 q-tiles x 4 key chunks of 2048 ----
            # The warm-up matmuls (PE p-state ramp while DMAs stream) and
            # the proto-similarity block live in ordinary ring slots: a
            # separate PSUM pool would serialize the pool transition, so
            # the first key matmul would wait for the proto exp. As ring
            # tiles they pipeline like any other chunk.
            with tc.tile_pool(name="ring", bufs=2, space="PSUM") as ring:
                rw = ring.tile([128, CHUNK], f32, tag="ps", name="rw")
                for _ in range(4):
                    nc.tensor.matmul(rw[:, 0:128], dmy[:], dmy[:],
                                     start=True, stop=True)
                # proto similarity transposed: [class, query] so it costs
                # one stationary load + two matmuls instead of eight
                rp = ring.tile([128, CHUNK], f32, tag="ps", name="rp")
                nc.tensor.matmul(rp[0:C, 0:512], protosT[:], kq[:, 0:512],
                                 start=True, stop=True)
                nc.tensor.matmul(rp[0:C, 512:Q], protosT[:], kq[:, 512:Q],
                                 start=True, stop=True)
                nc.scalar.activation(etpT[:], rp[0:C, 0:Q], Act.Exp,
                                     scale=1.0 / TAU)
                # proto selects on the idle GpSimd engine: one-hot
                # (numerator) and 1/cls_freq-weighted (denominator)
                # partition sums. The all-reduce writes through a permuted
                # AP so the [q=(t,p)] result lands in [p-major, t] memory
                # order; the reshape to [128, NQT] is then a contiguous
                # DMA partition scatter.
                nc.gpsimd.tensor_tensor(junkT[:], etpT[:], ohpT[:], op=mult)
                nc.gpsimd.partition_all_reduce(nprB[:].transpose([0, 2, 1]),
                                               junkT[:], channels=C,
                                               reduce_op=bass_isa.ReduceOp.add)
                nc.gpsimd.tensor_tensor(junkT[:], etpT[:], wB[:], op=mult)
                nc.gpsimd.partition_all_reduce(dprB[:].transpose([0, 2, 1]),
                                               junkT[:], channels=C,
                                               reduce_op=bass_isa.ReduceOp.add)
                nc.sync.dma_start(nprot[:], nprB[0:1, :, :])
                nc.sync.dma_start(dprot[:], dprB[0:1, :, :])

                for t in range(NQT):
                    for ch in range(NCH):
                        ps = ring.tile([128, CHUNK], f32, tag="ps")
                        for j in range(CHUNK // 512):
                            cb = CHUNK * ch + 512 * j
                            if cb < 512:
                                mv, off = kq0, cb
                            elif cb < 1024:
                                mv, off = kq1, cb - 512
                            elif cb < 2048:
                                mv, off = kq2, cb - 1024
                            elif cb < 4096:
                                mv, off = kB1, cb - 2048
                            elif cb < 6144:
                                mv, off = kB21, cb - 4096
                            else:
                                mv, off = kB22, cb - 6144
                            nc.tensor.matmul(ps[:, 512 * j:512 * (j + 1)],
                                             kq[:, 128 * t:128 * t + 128],
                                             mv[:, off:off + 512],
                                             start=True, stop=True)
                        et = etring.tile([128, CHUNK], bf16, tag="et")
                        # split row-sum work: 11 chunks via ACT accum_out
                        # (hidden RAA ~0.3us each; t==7 fully on ACT keeps
                        # DVE off the kernel tail), 21 via DVE reduces
                        on_act = (t == NQT - 1 or ch == NCH - 1
                                  or (t == NQT - 2 and ch == 2))
                        if on_act:
                            nc.scalar.activation(
                                et[:], ps[:], Act.Exp, scale=1.0 / TAU,
                                accum_out=racc[ch][:, t:t + 1])
                        else:
                            nc.scalar.activation(et[:], ps[:], Act.Exp,
                                                 scale=1.0 / TAU)
                        if ch == 0:
                            # band window: own-class sum (mask-mult + reduce
                            # on DVE; GpSimd owns the proto selects and an
                            # in-order cross-engine handoff here would stall
                            # the rowsum reduces behind it)
                            win = slice(64 + 128 * t, 64 + 128 * t + W)
                            nc.vector.tensor_tensor(junkw[:], et[:, win],
                                                    mclass[:, t], op=mult)
                            nc.vector.reduce_sum(own[:, t:t + 1], junkw[:],
                                                 axis=mybir.AxisListType.X)
                        if not on_act:
                            nc.vector.reduce_sum(racc[ch][:, t:t + 1], et[:],
                                                 axis=mybir.AxisListType.X)

            # ---- epilogue: dependency-staged so only the last few ops
            # trail the final exp chunk ----
            num = cst.tile([128, NQT], f32, tag="num")
            lbuf = cst.tile([128, NQT], f32, tag="lbuf")
            lbuf2 = cst.tile([128, NQT], f32, tag="lbuf2")
            l1 = cst.tile([128, 1], f32, tag="l1")
            l2 = cst.tile([128, 1], f32, tag="l2")
            # numerator path completes once t7ch0's band lands (early);
            # its reciprocal folds the two tail Lns into one:
            # sum(ln den - ln num) = sum(ln(den * (1/num)))
            rnum = cst.tile([128, NQT], f32, tag="rnum")
            nc.vector.tensor_tensor(num[:], own[:], nprot[:], op=add)
            nc.vector.reciprocal(rnum[:], num[:])

            # denominator row-sum partials fold in as each racc column set
            # completes; only "+ racc[3]" waits for the last RAA
            r01 = cst.tile([128, NQT], f32, tag="r01")
            r012 = cst.tile([128, NQT], f32, tag="r012")
            rs3p = cst.tile([128, NQT], f32, tag="rs3p")
            rs = cst.tile([128, NQT], f32, tag="rs")
            den = cst.tile([128, NQT], f32, tag="den")
            # denominator-side partials run on the idle GpSimd engine as
            # their dependencies land, instead of queueing behind DVE's
            # in-order rowsum backlog at the tail
            nc.gpsimd.tensor_tensor(r01[:], racc[0][:], racc[1][:], op=add)
            nc.gpsimd.tensor_tensor(r012[:], r01[:], racc[2][:], op=add)
            nc.gpsimd.tensor_tensor(rs3p[:], r012[:], dsubh[:], op=sub)
            nc.gpsimd.tensor_tensor(rs[:], rs3p[:], racc[3][:], op=add)
            nc.gpsimd.tensor_tensor(den[:], rs[:], fwinv[:], op=mult)
            nc.gpsimd.tensor_tensor(den[:], den[:], dprot[:], op=add)
            nc.vector.tensor_tensor(den[:], den[:], rnum[:], op=mult)
            nc.scalar.activation(lbuf2[:], den[:], Act.Ln, accum_out=l1[:])

            with tc.tile_pool(name="pf", bufs=1, space="PSUM") as pfp:
                pf = pfp.tile([1, 1], f32, tag="pf")
                nc.tensor.matmul(pf[:], onescol[:], l1[:],
                                 start=True, stop=True)
                res = cst.tile([1, 1], f32, tag="res")
                nc.vector.tensor_copy(res[:], pf[:])
                nc.sync.dma_start(d_out[:], res[:])

    nc.compile()
    return nc


def make_in_maps(protos, proj2, target2, proj3, target3):
    import ml_dtypes

    bf16 = ml_dtypes.bfloat16
    fp8 = ml_dtypes.float8_e4m3
    f32 = np.float32

    feats = np.concatenate([np.asarray(proj2, dtype=f32),
                            np.asarray(proj3, dtype=f32)], axis=0)
    labels = np.concatenate([np.asarray(target2), np.asarray(target3)],
                            axis=0).astype(np.int64)

    order = np.argsort(labels, kind="stable")
    fs = feats[order]
    ls = labels[order]
    nrm = np.sqrt((fs * fs).sum(axis=1, keepdims=True))
    fn = fs / np.maximum(nrm, f32(1e-12))

    counts = np.bincount(ls, minlength=C).astype(f32)
    # class-mates of any row must fit the [start-WMARG, end+WMARG] window
    assert counts.max() <= WMARG + 1, "class count exceeds band window"
    cls_freq = (counts + f32(1.0)) + f32(EPS_FREQ)
    cfr = (f32(1.0) / cls_freq).astype(f32)

    pr = np.asarray(protos, dtype=f32)
    pnrm = np.sqrt((pr * pr).sum(axis=1, keepdims=True))
    pn = pr / np.maximum(pnrm, f32(1e-12))
    protosT = np.ascontiguousarray(pn.T).astype(bf16)

    in_maps = []
    for c in range(N_CORES):
        roll = (Q * c - QOFF) % M
        idx = (np.arange(M) + roll) % M
        kf = fn[idx]
        kl = ls[idx]

        keysT = np.ascontiguousarray(kf.T).astype(bf16)
        keysT8 = np.ascontiguousarray(kf.T).astype(fp8)

        # host-side diagonal term: exp(q_bf16 . q_fp8 / tau) exactly as the
        # PE computes it (rounded inputs, f32 products/accumulate)
        q8 = keysT8[:, QOFF:QOFF + Q].astype(np.float64)
        ss = (q8 * q8).sum(axis=0)                       # [Q]
        dsubh = np.exp(ss * (1.0 / TAU)).astype(f32)     # [Q]
        dsubh = np.ascontiguousarray(dsubh.reshape(NQT, 128).T)  # [128, NQT]

        mcls = np.zeros((128, NQT, W), dtype=bf16)
        fwinv = np.zeros((128, NQT), dtype=f32)
        for t in range(NQT):
            rows = kl[QOFF + 128 * t:QOFF + 128 * t + 128]
            win = kl[64 + 128 * t:64 + 128 * t + W]
            mc = rows[:, None] == win[None, :]
            mc[np.arange(128), np.arange(128) + WMARG] = False
            mcls[:, t, :] = mc.astype(bf16)
            fwinv[:, t] = cfr[rows]

        # transposed proto-select masks [class, query]
        qlab = kl[QOFF:QOFF + Q]                          # [Q]
        ohpT = (np.arange(C)[:, None] == qlab[None, :]).astype(bf16)
        wB = np.ascontiguousarray(
            np.broadcast_to(cfr[:, None], (C, Q))).astype(bf16)

        protosT8 = np.ascontiguousarray(pn.T).astype(fp8)
        pkq = np.concatenate([protosT8, keysT8[:, QOFF:QOFF + Q]], axis=1)
        im = {
            "pkq": np.ascontiguousarray(pkq),
            "kq0": np.ascontiguousarray(keysT8[:, 0:512]),
            "kq1": np.ascontiguousarray(keysT8[:, 512:1024]),
            "kq2": np.ascontiguousarray(keysT8[:, 1024:2048]),
            "kB1": np.ascontiguousarray(keysT8[:, 2048:4096]),
            "kB21": np.ascontiguousarray(keysT8[:, 4096:6144]),
            "kB22": np.ascontiguousarray(keysT8[:, 6144:8192]),
            "mclass": mcls,
            "fwinv": fwinv,
            "dsubh": dsubh,
            "ohpT": np.ascontiguousarray(ohpT),
            "wB": wB,
        }
        in_maps.append(im)
    return in_maps


def run(in_maps, trace=False):
    _install_ntff_hook()
    from concourse import bass_utils

    nc = build_nc()
    res = bass_utils.run_bass_kernel_spmd(
        nc, in_maps, core_ids=list(range(N_CORES)), trace=trace)
    return res


def kernel(protos, proj2, target2, proj3, target3):
    in_maps = make_in_maps(protos, proj2, target2, proj3, target3)
    res = run(in_maps, trace=False)
    parts = [res.results[i]["out"][0, 0] for i in range(N_CORES)]
    total = np.sum(np.asarray(parts, dtype=np.float32))
    return np.asarray(total / np.float32(M), dtype=np.float32)
